# revision 13
# baseline (speedup 1.0000x reference)
"""HRA (Householder Reflection Adaptation) forward kernel for Trainium2.

Math: out = x @ Q with Q = prod_i (I - 2 u_i u_i^T), u_i = normalized columns
of hra_u [4096, 8].  Using the compact WY representation:
    Q = I - U T U^T      (T upper-triangular 8x8, diag=2)
    out = x - (x @ A) @ U^T,   A = U @ T

Sharding: data-parallel over rows. x [4,2048,4096] -> [8192, 4096]; each of
8 cores gets 1024 contiguous rows.

Layout/precision strategy (the whole kernel is HBM-bandwidth-bound):
  * device I/O is fp16 (tolerance is 2e-2; fp16 keeps rel err ~1e-4),
    halving HBM traffic vs f32: 16.8 MB/core round trip ~= 47 us floor.
  * the host uploads x TRANSPOSED (x^T [4096, 1024] per core), so the
    projection matmuls P^T[8,r] += A_c^T @ xT_c run directly on DMA'd
    tiles -- no PE transposes and no PSUM->SBUF copy pass at all.
  * updates stay transposed: outT_c = xT_c - U_c @ P^T, subtracted in
    place in SBUF; the host transposes the fp16 result back.

Pipeline: rows are split in two halves pipelined against each other to
keep the DMA engines streaming continuously and to spread compute-engine
duty (the TRN2 power governor clamps utilization to ~50% if too many
engines run hot simultaneously -- a concentrated all-engine tail phase
trips it):
  in-DMA half0 | proj h0 | in-DMA half1 + (upd/sub/out-DMA h0) | tail h1
DMA triggers are batched 4 chunks (512 KB) apiece to keep the SP
sequencer (~650 ns per trigger) off the critical path.  The f32-PSUM ->
fp16-SBUF subtract alternates DVE direct psum-subs with ACT convert +
DVE fp16-sub so both engines stay ~55% duty, under the governor's trip
point.
"""

import os
import sys

for _p in ("/opt/trn_rl_repo", "/root/.axon_site", "/root/.axon_site/_ro/trn_rl_repo",
           "/root/.axon_site/_ro/pypackages"):
    if os.path.isdir(_p) and _p not in sys.path:
        sys.path.append(_p)

import numpy as np

import concourse.bass as bass
import concourse.mybir as mybir
import concourse.tile as tile
from concourse import bacc
from concourse.bass_utils import run_bass_kernel_spmd

B, S, D, R = 4, 2048, 4096, 8
N_CORES = 8
ROWS = B * S                      # 8192
RPC = ROWS // N_CORES             # 1024 rows per core
P = 128
DC = D // P                       # 32 d-chunks
H = RPC // 2                      # 512 rows per half (PSUM bank f32 size)
G = 4                             # chunks per DMA trigger
NG = DC // G                      # 8 groups per half

F32 = mybir.dt.float32
F16 = mybir.dt.float16

_CACHE = {}


def _householder_wy(hra_u: np.ndarray):
    """Return (A, UT) with out = x - (x @ A) @ UT."""
    u = hra_u.astype(np.float32)
    u = u / np.linalg.norm(u, axis=0, keepdims=True)
    U = u.astype(np.float64)
    T = np.zeros((R, R), np.float64)
    for k in range(R):
        T[k, k] = 2.0
        if k:
            T[:k, k] = -2.0 * (T[:k, :k] @ (U[:, :k].T @ U[:, k]))
    A = (U @ T).astype(np.float32)          # [D, R]
    return A, np.ascontiguousarray(u.T)     # [R, D]


def _build_program():
    nc = bacc.Bacc(trn_type="TRN2")
    xt = nc.dram_tensor("xt", (D, RPC), F16, kind="ExternalInput")
    a = nc.dram_tensor("a", (P, DC * R), F16, kind="ExternalInput")
    ut = nc.dram_tensor("ut", (R, D), F16, kind="ExternalInput")
    out = nc.dram_tensor("out", (D, RPC), F16, kind="ExternalOutput")

    # [partition, chunk, half, row] -- partition-major to match SBUF APs
    xtd = xt.rearrange("(c p) (h r) -> p c h r", p=P, h=2)
    otd = out.rearrange("(c p) (h r) -> p c h r", p=P, h=2)

    with tile.TileContext(nc) as tc:
        with (
            tc.tile_pool(name="const", bufs=1) as const,
            tc.tile_pool(name="upd", bufs=4) as upd_pool,
            tc.tile_pool(name="psp", bufs=1, space="PSUM") as psp_pool,
            tc.tile_pool(name="pso", bufs=5, space="PSUM") as pso_pool,
        ):
            a_sb = const.tile([P, DC * R], F16)
            nc.sync.dma_start(a_sb, a[:, :])
            ut_sb = const.tile([R, D], F16)
            nc.sync.dma_start(ut_sb, ut[:, :])

            xall = const.tile([P, DC, 2, H], F16)

            def dma_in(h):
                for g in range(NG):
                    nc.sync.dma_start(xall[:, g * G:(g + 1) * G, h, :],
                                      xtd[:, g * G:(g + 1) * G, h, :])

            dma_in(0)

            # PE warm-up: observe each const DMA once (one sync-wait per
            # LDWEIGHTS) and run a few us of matmuls so the PE p-state
            # ramps while the first DMA fill runs.
            warm = pso_pool.tile([P, H], F32, tag="ps_o")
            nc.tensor.matmul(warm[:R, :256], a_sb[:, :R], a_sb[:, :256],
                             start=True, stop=True)
            for _ in range(10):
                nc.tensor.matmul(warm, ut_sb[:, :P], ut_sb[:, :H],
                                 start=True, stop=True)

            dma_in(1)

            ps_p = psp_pool.tile([R, 2, H], F32, tag="ps_p")
            pt = const.tile([R, 2, H], F16)

            def proj_mm(h, c):
                # P^T[8, H] += A_c^T @ xT_c,h  accumulated over chunks
                nc.tensor.matmul(
                    ps_p[:, h, :],
                    a_sb[:, c * R:(c + 1) * R],
                    xall[:, c, h, :],
                    start=(c == 0),
                    stop=(c == DC - 1),
                )

            def tail_unit(h, c):
                # outT_c = xT_c - U_c @ P^T in place.  Consumer rotation
                # V/B/V/C keeps each of DVE/ACT/Pool under ~50% duty so
                # the power governor stays at k=8.
                ps_o = pso_pool.tile([P, H], F32, tag="ps_o")
                nc.tensor.matmul(
                    ps_o,
                    ut_sb[:, c * P:(c + 1) * P],
                    pt[:, h, :],
                    start=True,
                    stop=True,
                )
                xc = xall[:, c, h, :]
                if c % 16 in (0, 3, 6, 9, 13):
                    nc.vector.tensor_sub(xc, xc, ps_o)
                else:
                    u_sb = upd_pool.tile([P, H], F16, tag="upd")
                    nc.scalar.copy(u_sb, ps_o)
                    nc.vector.tensor_sub(xc, xc, u_sb)

            def dma_out(h, g):
                nc.sync.dma_start(otd[:, g * G:(g + 1) * G, h, :],
                                  xall[:, g * G:(g + 1) * G, h, :])

            for c in range(DC):
                proj_mm(0, c)
            nc.vector.tensor_copy(pt[:, 0, :], ps_p[:, 0, :])

            # middle: drain half-0 while half-1 streams in; PE alternates
            # upd(h0) and proj(h1) matmuls so both make progress at the
            # DMA arrival rate.
            for g in range(NG):
                for i in range(G):
                    c = g * G + i
                    tail_unit(0, c)
                    proj_mm(1, c)
                dma_out(0, g)
            nc.vector.tensor_copy(pt[:, 1, :], ps_p[:, 1, :])

            for g in range(NG):
                for i in range(G):
                    tail_unit(1, g * G + i)
                dma_out(1, g)

    nc.compile()
    return nc


def _get_program():
    if "nc" not in _CACHE:
        _CACHE["nc"] = _build_program()
    return _CACHE["nc"]


def kernel(input, hra_u, **run_kwargs):
    input = np.asarray(input, dtype=np.float32)
    hra_u = np.asarray(hra_u, dtype=np.float32)

    A, UT = _householder_wy(hra_u)
    # pack A [D, R] so partition p holds A[c*128+p, :] at free offset c*R
    a_packed = np.ascontiguousarray(
        A.reshape(DC, P, R).transpose(1, 0, 2).reshape(P, DC * R)
    ).astype(np.float16)
    ut_f16 = UT.astype(np.float16)

    x_flat = input.reshape(ROWS, D)
    in_maps = [
        {
            "xt": x_flat[c * RPC:(c + 1) * RPC].T.astype(np.float16),
            "a": a_packed,
            "ut": ut_f16,
        }
        for c in range(N_CORES)
    ]

    nc = _get_program()
    res = run_bass_kernel_spmd(nc, in_maps, core_ids=list(range(N_CORES)),
                               **run_kwargs)
    out = np.empty((ROWS, D), dtype=np.float32)
    for c in range(N_CORES):
        out[c * RPC:(c + 1) * RPC] = res.results[c]["out"].T
    if run_kwargs:
        kernel.last_results = res
    return out.reshape(B, S, D)


# revision 15
# speedup vs baseline: 1.1541x; 1.1541x over previous
"""HRA (Householder Reflection Adaptation) forward kernel for Trainium2.

Math: out = x @ Q with Q = prod_i (I - 2 u_i u_i^T), u_i = normalized columns
of hra_u [4096, 8].  Using the compact WY representation:
    Q = I - U T U^T      (T upper-triangular 8x8, diag=2)
    out = x - (x @ A) @ U^T,   A = U @ T

Sharding: data-parallel over rows. x [4,2048,4096] -> [8192, 4096]; each of
8 cores gets 1024 contiguous rows.

Layout/precision strategy (HBM-bandwidth / PE-instruction bound):
  * device I/O is fp16 (tolerance is 2e-2; fp16 keeps rel err ~1e-4),
    halving HBM traffic vs f32: 16.8 MB/core round trip.
  * the host uploads x TRANSPOSED (x^T [4096, 1024] per core), so the
    projection matmuls P^T[8,r] += A_c^T @ xT_c run directly on DMA'd
    tiles -- no PE transposes and no PSUM->SBUF copy pass at all.
  * updates stay transposed: outT_c = xT_c - U_c @ P^T, subtracted in
    place in SBUF; the host transposes the fp16 result back.

Schedule (monolithic; input phase = DMA+PE only, then the tail):
  32 chunk DMAs stream in while the PE accumulates P^T; the tail then
  runs per chunk: 2 update matmuls -> PSUM, drain to fp16 SBUF rotated
  across DVE (direct psum-subtract), ACT convert + DVE fp16-subtract,
  and ACT convert + Pool fp16-subtract, then a per-chunk DMA-out.
  Out-triggers are emitted per 4-chunk group ordered fast-drains-first
  so a slow Pool drain never head-of-line blocks the SP trigger queue.
  The phase boundary is kept tight: tiny warm-up (PE ramp only), pt
  copied in halves so the first update matmul starts ~0.6us earlier.
"""

import os
import sys

for _p in ("/opt/trn_rl_repo", "/root/.axon_site", "/root/.axon_site/_ro/trn_rl_repo",
           "/root/.axon_site/_ro/pypackages"):
    if os.path.isdir(_p) and _p not in sys.path:
        sys.path.append(_p)

import numpy as np

import concourse.bass as bass
import concourse.mybir as mybir
import concourse.tile as tile
from concourse import bacc
from concourse.bass_utils import run_bass_kernel_spmd

B, S, D, R = 4, 2048, 4096, 8
N_CORES = 8
ROWS = B * S                      # 8192
RPC = ROWS // N_CORES             # 1024 rows per core
P = 128
DC = D // P                       # 32 d-chunks
H = RPC // 2                      # 512 rows per half (PSUM bank f32 size)

F32 = mybir.dt.float32
F16 = mybir.dt.float16

_CACHE = {}


def _householder_wy(hra_u: np.ndarray):
    """Return (A, UT) with out = x - (x @ A) @ UT."""
    u = hra_u.astype(np.float32)
    u = u / np.linalg.norm(u, axis=0, keepdims=True)
    U = u.astype(np.float64)
    T = np.zeros((R, R), np.float64)
    for k in range(R):
        T[k, k] = 2.0
        if k:
            T[:k, k] = -2.0 * (T[:k, :k] @ (U[:, :k].T @ U[:, k]))
    A = (U @ T).astype(np.float32)          # [D, R]
    return A, np.ascontiguousarray(u.T)     # [R, D]


# tail drain rotation per 8 chunks: 0=DVE direct psum-sub,
# 1=ACT convert + DVE fp16 sub, 2=ACT convert + Pool fp16 sub
_TAIL = [0, 1, 2, 0, 1, 0, 1, 2]
# out-trigger order within each 8-chunk block: fast drains first so the
# in-order SP trigger queue is never blocked by a slow Pool drain
_TRIG = [0, 3, 5, 1, 4, 6, 2, 7]


def _build_program():
    nc = bacc.Bacc(trn_type="TRN2")
    xt = nc.dram_tensor("xt", (D, RPC), F16, kind="ExternalInput")
    a = nc.dram_tensor("a", (P, DC * R), F16, kind="ExternalInput")
    ut = nc.dram_tensor("ut", (R, D), F16, kind="ExternalInput")
    out = nc.dram_tensor("out", (D, RPC), F16, kind="ExternalOutput")

    xtd = xt.rearrange("(c p) r -> p c r", p=P)   # [128, DC, RPC]
    otd = out.rearrange("(c p) r -> p c r", p=P)

    with tile.TileContext(nc) as tc:
        with (
            tc.tile_pool(name="const", bufs=1) as const,
            tc.tile_pool(name="upd", bufs=4) as upd_pool,
            tc.tile_pool(name="psp", bufs=1, space="PSUM") as psp_pool,
            tc.tile_pool(name="pso", bufs=3, space="PSUM") as pso_pool,
        ):
            a_sb = const.tile([P, DC * R], F16)
            nc.sync.dma_start(a_sb, a[:, :])
            ut_sb = const.tile([R, D], F16)
            nc.sync.dma_start(ut_sb, ut[:, :])

            xall = const.tile([P, DC, RPC], F16)
            nc.sync.dma_start(xall[:, 0, :], xtd[:, 0, :])

            # tiny PE warm-up: observe each const DMA once (one sync-wait
            # per LDWEIGHTS) and keep the PE awake during the DMA fill
            # without delaying the first projection matmuls.
            warm = pso_pool.tile([P, 2, H], F32, tag="ps_o")
            nc.tensor.matmul(warm[:R, 0, :256], a_sb[:, :R], a_sb[:, :256],
                             start=True, stop=True)
            for _ in range(4):
                nc.tensor.matmul(warm[:, 0, :P], ut_sb[:, :P], ut_sb[:, :P],
                                 start=True, stop=True)

            for c in range(1, DC):
                nc.sync.dma_start(xall[:, c, :], xtd[:, c, :])

            ps_p = psp_pool.tile([R, 2, H], F32, tag="ps_p")
            pt = const.tile([R, 2, H], F16)

            # projection: P^T[8, RPC] += A_c^T @ xT_c, PSUM-accumulated
            for c in range(DC):
                for h in range(2):
                    nc.tensor.matmul(
                        ps_p[:, h, :],
                        a_sb[:, c * R:(c + 1) * R],
                        xall[:, c, h * H:(h + 1) * H],
                        start=(c == 0),
                        stop=(c == DC - 1),
                    )
            # pt in halves so the first update matmul starts sooner
            nc.vector.tensor_copy(pt[:, 0, :], ps_p[:, 0, :])
            nc.vector.tensor_copy(pt[:, 1, :], ps_p[:, 1, :])

            # tail: outT_c = xT_c - U_c @ P^T in place, then DMA out
            done = {}
            for c in range(DC):
                ps_o = pso_pool.tile([P, 2, H], F32, tag="ps_o")
                for h in range(2):
                    nc.tensor.matmul(
                        ps_o[:, h, :],
                        ut_sb[:, c * P:(c + 1) * P],
                        pt[:, h, :],
                        start=True,
                        stop=True,
                    )
                xc = xall[:, c, :]
                kind = _TAIL[c % 8]
                if kind == 0:
                    nc.vector.tensor_sub(xc, xc, ps_o)
                else:
                    u_sb = upd_pool.tile([P, 2, H], F16, tag="upd")
                    nc.scalar.copy(u_sb, ps_o)
                    if kind == 1:
                        nc.vector.tensor_sub(xc, xc, u_sb)
                    else:
                        nc.gpsimd.tensor_sub(xc, xc, u_sb)
                done[c] = True
                # flush triggers per 4-block, fastest drains first, so a
                # slow Pool drain never head-of-line blocks the in-order
                # SP trigger queue
                if c % 4 == 3:
                    block = list(range(c - 3, c + 1))
                    for cc in sorted(block, key=lambda q: _TAIL[q % 8]):
                        nc.sync.dma_start(otd[:, cc, :], xall[:, cc, :])

    nc.compile()
    return nc


def _get_program():
    if "nc" not in _CACHE:
        _CACHE["nc"] = _build_program()
    return _CACHE["nc"]


def kernel(input, hra_u, **run_kwargs):
    input = np.asarray(input, dtype=np.float32)
    hra_u = np.asarray(hra_u, dtype=np.float32)

    A, UT = _householder_wy(hra_u)
    # pack A [D, R] so partition p holds A[c*128+p, :] at free offset c*R
    a_packed = np.ascontiguousarray(
        A.reshape(DC, P, R).transpose(1, 0, 2).reshape(P, DC * R)
    ).astype(np.float16)
    ut_f16 = UT.astype(np.float16)

    x_flat = input.reshape(ROWS, D)
    in_maps = [
        {
            "xt": x_flat[c * RPC:(c + 1) * RPC].T.astype(np.float16),
            "a": a_packed,
            "ut": ut_f16,
        }
        for c in range(N_CORES)
    ]

    nc = _get_program()
    res = run_bass_kernel_spmd(nc, in_maps, core_ids=list(range(N_CORES)),
                               **run_kwargs)
    out = np.empty((ROWS, D), dtype=np.float32)
    for c in range(N_CORES):
        out[c * RPC:(c + 1) * RPC] = res.results[c]["out"].T
    if run_kwargs:
        kernel.last_results = res
    return out.reshape(B, S, D)
